# revision 6
# baseline (speedup 1.0000x reference)
"""NonLocalBlock (B=8, C=64, H=W=64) on 8 Trainium2 NeuronCores.

Strategy: data-parallel over batch — core b handles batch element b fully
(no collectives). Per core, a flash-style attention over N=4096 tokens:

  xf1  = [x_b ; ones]                        [65, 4096]  (ones row folds biases)
  thphi = wproj^T @ xf1                      [128, 4096] rows 0..95 = theta
          (theta replicated 3x across partition groups for PE row tiling,
           rows 96..127 = phi)
  phi3: phi m-tiles rearranged so row-tile i holds m-tile 3g+i at
        partitions 32i..32i+32 (SBUF->SBUF DMA)
  gt   = G'^T tiles: G' = [w_o@w_g x + w_o@b_g + b_o ; ones]   [128, 65] x 32
         (ones column makes the PV matmul also produce the softmax denom)
  for each n-chunk (512 queries):
     for each m-group (3 m-tiles): S^T = phi_tile^T theta  (3x row-tiled K=32)
                                   P^T = exp(S^T)           (ScalarE, PSUM->SBUF)
                                   po += gt_m^T P^T         (K=128 accumulate)
     out = po[0:64] * (1 / po[64]) broadcast    (recip + PE bcast + DVE mult)

All matmuls bf16 (1 cyc/col on PE; fp32 would be 4x slower), accumulation and
softmax math fp32.
"""

import numpy as np
import ml_dtypes

BF16 = ml_dtypes.bfloat16

B, C, H, W = 8, 64, 64, 64
N = H * W          # 4096 tokens
CI = C // 2        # 32 intermediate channels
P = 128
NCHUNK = 512       # queries per outer chunk
NT = N // NCHUNK   # 8 chunks
MT = N // P        # 32 key tiles of 128
RT = 3             # row-tiling width (m-tiles per group)
NG = (MT + RT - 1) // RT  # 11 groups (last has 2)

_cache = {}


def _build_program():
    import concourse.mybir as mybir
    import concourse.tile as tile
    from concourse import bacc

    f32 = mybir.dt.float32
    bf16 = mybir.dt.bfloat16
    EXP = mybir.ActivationFunctionType.Exp

    nc = bacc.Bacc()
    xf1_d = nc.declare_dram_parameter("xf1", [C + 1, N], bf16, isOutput=False)
    wproj_d = nc.declare_dram_parameter("wproj", [C + 1, P], bf16, isOutput=False)
    w2p_d = nc.declare_dram_parameter("w2p", [C + 1, C + 1], bf16, isOutput=False)
    y_d = nc.declare_dram_parameter("y", [C, N], f32, isOutput=True)

    with tile.TileContext(nc) as tc:
        with (
            tc.tile_pool(name="const", bufs=1) as const_pool,
            tc.tile_pool(name="pt", bufs=3) as pt_pool,
            tc.tile_pool(name="outsb", bufs=2) as out_pool,
            tc.tile_pool(name="small", bufs=2) as small_pool,
        ):
            xf1 = const_pool.tile([C + 1, N], bf16)
            nc.sync.dma_start(xf1[:], xf1_d[:])
            wproj = const_pool.tile([C + 1, P], bf16)
            nc.sync.dma_start(wproj[:], wproj_d[:])
            w2p = const_pool.tile([C + 1, C + 1], bf16)
            nc.sync.dma_start(w2p[:], w2p_d[:])
            onesb = const_pool.tile([1, C], bf16)
            nc.any.memset(onesb[:], 1.0)

            thphi = const_pool.tile([P, N], bf16)
            phi3 = const_pool.tile([96, NG * P], bf16)
            gt = const_pool.tile([P, MT * (C + 1)], bf16)

            # --- projection phase: own PSUM pool, released before attention ---
            with tc.tile_pool(name="mpsum", bufs=2, space="PSUM") as mpsum_pool:
                # thphi[0:96] = theta x3 replicas, [96:128] = phi
                for t in range(NT):
                    ps = mpsum_pool.tile([P, NCHUNK], f32, tag="mp")
                    nc.tensor.matmul(
                        ps[:], wproj[:], xf1[:, t * NCHUNK:(t + 1) * NCHUNK],
                        start=True, stop=True,
                    )
                    nc.vector.tensor_copy(
                        thphi[:, t * NCHUNK:(t + 1) * NCHUNK], ps[:]
                    )
                # gt: G'^T m-tiles [128, 65] each
                for m in range(MT):
                    psg = mpsum_pool.tile([P, NCHUNK], f32, tag="mp")
                    nc.tensor.matmul(
                        psg[:, :C + 1], xf1[:, m * P:(m + 1) * P], w2p[:],
                        start=True, stop=True,
                    )
                    nc.vector.tensor_copy(
                        gt[:, m * (C + 1):(m + 1) * (C + 1)], psg[:, :C + 1]
                    )

            # phi3: row-tile i gets phi m-tile (3g+i) at partitions 32i..32i+32
            for m in range(MT):
                g, i = divmod(m, RT)
                nc.sync.dma_start(
                    phi3[32 * i:32 * (i + 1), g * P:(g + 1) * P],
                    thphi[96:128, m * P:(m + 1) * P],
                )

            # --- attention ---
            with (
                tc.tile_pool(name="st", bufs=2, space="PSUM") as st_pool,
                tc.tile_pool(name="po", bufs=2, space="PSUM") as po_pool,
            ):
                for t in range(NT):
                    nsl = slice(t * NCHUNK, (t + 1) * NCHUNK)
                    # pob[0:65]: PV accumulator (rows 0..63 = out, row 64 = l)
                    # pob[64:128]: 1/l broadcast, written later via col-tiled mm
                    pob = po_pool.tile([P, NCHUNK], f32)
                    for g in range(NG):
                        w = min(RT, MT - g * RT)
                        st = st_pool.tile([P, RT * NCHUNK], f32)
                        for i in range(w):
                            nc.tensor.matmul(
                                st[:, i * NCHUNK:(i + 1) * NCHUNK],
                                phi3[32 * i:32 * (i + 1), g * P:(g + 1) * P],
                                thphi[32 * i:32 * (i + 1), nsl],
                                start=True, stop=True,
                                tile_position=(32 * i, 0),
                            )
                        pt = pt_pool.tile([P, RT * NCHUNK], bf16)
                        nc.scalar.activation(
                            pt[:, :w * NCHUNK], st[:, :w * NCHUNK], EXP
                        )
                        for i in range(w):
                            m = g * RT + i
                            nc.tensor.matmul(
                                pob[:C + 1, :],
                                gt[:, m * (C + 1):(m + 1) * (C + 1)],
                                pt[:, i * NCHUNK:(i + 1) * NCHUNK],
                                start=(m == 0), stop=(m == MT - 1),
                                skip_group_check=True,
                            )
                    # normalize: y = pob[0:64] / pob[64]
                    linv = small_pool.tile([1, NCHUNK], bf16, tag="linv")
                    with nc.allow_low_precision(
                        reason="softmax denom bf16: 0.4% scale err, gate 2e-2"
                    ):
                        nc.vector.reciprocal(linv[:], pob[C:C + 1, :])
                    nc.tensor.matmul(
                        pob[C:, :], onesb[:], linv[:], start=True, stop=True,
                        tile_position=(0, C), skip_group_check=True,
                    )
                    lbs = out_pool.tile([C, NCHUNK], f32, tag="lbs")
                    nc.vector.tensor_copy(lbs[:], pob[C:, :])
                    outsb = out_pool.tile([C, NCHUNK], f32)
                    nc.vector.tensor_mul(outsb[:], pob[:C, :], lbs[:])
                    nc.sync.dma_start(y_d[:, nsl], outsb[:])

    nc.compile()
    return nc


def _prep_weights(w_g, b_g, w_theta, b_theta, w_phi, b_phi, w_o, b_o):
    # wproj [65, 128]: cols 32i+c (i<3) -> theta row c; cols 96+c -> phi row c.
    # Row 64 multiplies the ones-row of xf1, i.e. carries the bias.
    wproj = np.zeros((C + 1, P), np.float32)
    for i in range(RT):
        wproj[:C, 32 * i:32 * (i + 1)] = w_theta.T
        wproj[C, 32 * i:32 * (i + 1)] = b_theta
    wproj[:C, 96:128] = w_phi.T
    wproj[C, 96:128] = b_phi

    # w2p [65, 65]: G' = [(w_o w_g) x + (w_o b_g + b_o); ones]
    w2 = w_o.astype(np.float64) @ w_g.astype(np.float64)
    bias2 = w_o.astype(np.float64) @ b_g.astype(np.float64) + b_o
    w2p = np.zeros((C + 1, C + 1), np.float32)
    w2p[:C, :C] = w2.T
    w2p[C, :C] = bias2
    w2p[C, C] = 1.0
    return wproj.astype(BF16), w2p.astype(BF16)


def kernel(x, w_g, b_g, w_theta, b_theta, w_phi, b_phi, w_o, b_o):
    import sys
    if "/opt/trn_rl_repo" not in sys.path:
        sys.path.insert(0, "/opt/trn_rl_repo")
    from concourse.bass_utils import run_bass_kernel_spmd

    if "nc" not in _cache:
        _cache["nc"] = _build_program()
    nc = _cache["nc"]

    wproj, w2p = _prep_weights(
        w_g, b_g, w_theta, b_theta, w_phi, b_phi, w_o, b_o
    )
    in_maps = []
    for b in range(B):
        xf1 = np.ones((C + 1, N), np.float32)
        xf1[:C] = np.asarray(x[b], np.float32).reshape(C, N)
        in_maps.append({
            "xf1": xf1.astype(BF16),
            "wproj": wproj,
            "w2p": w2p,
        })

    res = run_bass_kernel_spmd(nc, in_maps, core_ids=list(range(B)))
    out = np.stack([res.results[b]["y"].reshape(C, H, W) for b in range(B)])
    return np.ascontiguousarray(out.astype(np.float32))


if __name__ == "__main__":
    rng = np.random.default_rng(0)
    inputs = {
        "x": rng.normal(size=(B, C, H, W)).astype(np.float32),
        "w_g": (rng.normal(size=(CI, C)) * 0.05).astype(np.float32),
        "b_g": (rng.normal(size=(CI,)) * 0.05).astype(np.float32),
        "w_theta": (rng.normal(size=(CI, C)) * 0.05).astype(np.float32),
        "b_theta": (rng.normal(size=(CI,)) * 0.05).astype(np.float32),
        "w_phi": (rng.normal(size=(CI, C)) * 0.05).astype(np.float32),
        "b_phi": (rng.normal(size=(CI,)) * 0.05).astype(np.float32),
        "w_o": (rng.normal(size=(C, CI)) * 0.05).astype(np.float32),
        "b_o": (rng.normal(size=(C,)) * 0.05).astype(np.float32),
    }
    y = kernel(**inputs)
    print(y.shape, y.dtype)


# revision 8
# speedup vs baseline: 1.2838x; 1.2838x over previous
"""NonLocalBlock (B=8, C=64, H=W=64) on 8 Trainium2 NeuronCores.

Strategy: data-parallel over batch — core b handles batch element b fully
(no collectives). Per core, a flash-style attention over N=4096 tokens:

  xf1  = [x_b ; ones]                        [65, 4096]  (ones row folds biases)
  thphi = wproj^T @ xf1                      [128, 4096] rows 0..95 = theta
          (theta replicated 3x across partition groups for PE row tiling,
           rows 96..127 = phi)
  phi3: phi m-tiles rearranged so row-tile i holds m-tile 3g+i at
        partitions 32i..32i+32 (SBUF->SBUF DMA)
  gt   = G'^T tiles: G' = [w_o@w_g x + w_o@b_g + b_o ; ones]   [128, 65] x 32
         (ones column makes the PV matmul also produce the softmax denom)
  for each n-chunk (512 queries):
     for each m-group (3 m-tiles): S^T = phi_tile^T theta  (3x row-tiled K=32)
                                   P^T = exp(S^T)           (ScalarE, PSUM->SBUF)
                                   po += gt_m^T P^T         (K=128 accumulate)
     out = po[0:64] * (1 / po[64]) broadcast    (recip + PE bcast + DVE mult)

All matmuls bf16 (1 cyc/col on PE; fp32 would be 4x slower), accumulation and
softmax math fp32.
"""

import numpy as np
import ml_dtypes

BF16 = ml_dtypes.bfloat16

B, C, H, W = 8, 64, 64, 64
N = H * W          # 4096 tokens
CI = C // 2        # 32 intermediate channels
P = 128
NCHUNK = 512       # queries per outer chunk
NT = N // NCHUNK   # 8 chunks
MT = N // P        # 32 key tiles of 128
RT = 3             # row-tiling width (m-tiles per group)
NG = (MT + RT - 1) // RT  # 11 groups (last has 2)

_cache = {}


def _build_program():
    import concourse.mybir as mybir
    import concourse.tile as tile
    from concourse import bacc

    f32 = mybir.dt.float32
    bf16 = mybir.dt.bfloat16
    EXP = mybir.ActivationFunctionType.Exp

    nc = bacc.Bacc()
    xf1_d = nc.declare_dram_parameter("xf1", [C + 1, N], bf16, isOutput=False)
    wproj_d = nc.declare_dram_parameter("wproj", [C + 1, P], bf16, isOutput=False)
    w2p_d = nc.declare_dram_parameter("w2p", [C + 1, C + 1], bf16, isOutput=False)
    y_d = nc.declare_dram_parameter("y", [C, N], f32, isOutput=True)

    with tile.TileContext(nc) as tc:
        with (
            tc.tile_pool(name="const", bufs=1) as const_pool,
            tc.tile_pool(name="pt", bufs=3) as pt_pool,
            tc.tile_pool(name="outsb", bufs=2) as out_pool,
            tc.tile_pool(name="small", bufs=2) as small_pool,
            tc.tile_pool(name="st", bufs=2, space="PSUM") as st_pool,
            tc.tile_pool(name="po", bufs=2, space="PSUM") as po_pool,
        ):
            xf1 = const_pool.tile([C + 1, N], bf16)
            wproj = const_pool.tile([C + 1, P], bf16)
            nc.sync.dma_start(wproj[:], wproj_d[:])
            w2p = const_pool.tile([C + 1, C + 1], bf16)
            nc.sync.dma_start(w2p[:], w2p_d[:])
            onesb = const_pool.tile([1, C], bf16)
            nc.any.memset(onesb[:], 1.0)

            thphi = const_pool.tile([P, N], bf16)
            phi3 = const_pool.tile([96, NG * P], bf16)
            gt = const_pool.tile([P, MT * (C + 1)], bf16)

            # projections: thphi[0:96] = theta x3 replicas, [96:128] = phi.
            # xf1 DMA'd per chunk so the first matmul starts early.
            for t in range(NT):
                csl = slice(t * NCHUNK, (t + 1) * NCHUNK)
                nc.sync.dma_start(xf1[:, csl], xf1_d[:, csl])
                ps = po_pool.tile([P, NCHUNK], f32, tag="pob")
                nc.tensor.matmul(ps[:], wproj[:], xf1[:, csl],
                                 start=True, stop=True)
                nc.vector.tensor_copy(thphi[:, csl], ps[:])
                # phi3 rearrange: row-tile i gets phi m-tile (3g+i) at
                # partitions 32i..32i+32
                for m in range(4 * t, 4 * (t + 1)):
                    g, i = divmod(m, RT)
                    nc.sync.dma_start(
                        phi3[32 * i:32 * (i + 1), g * P:(g + 1) * P],
                        thphi[96:128, m * P:(m + 1) * P],
                    )

            def st_exp_group(t, g):
                w = min(RT, MT - g * RT)
                st = st_pool.tile([P, RT * NCHUNK], f32)
                for i in range(w):
                    nc.tensor.matmul(
                        st[:, i * NCHUNK:(i + 1) * NCHUNK],
                        phi3[32 * i:32 * (i + 1), g * P:(g + 1) * P],
                        thphi[32 * i:32 * (i + 1),
                              t * NCHUNK:(t + 1) * NCHUNK],
                        start=True, stop=True,
                        tile_position=(32 * i, 0),
                    )
                pt = pt_pool.tile([P, RT * NCHUNK], bf16)
                nc.scalar.activation(pt[:, :w * NCHUNK], st[:, :w * NCHUNK],
                                     EXP)
                return pt

            def pv_group(t, g, pob, pt):
                w = min(RT, MT - g * RT)
                for i in range(w):
                    m = g * RT + i
                    nc.tensor.matmul(
                        pob[:C + 1, :],
                        gt[:, m * (C + 1):(m + 1) * (C + 1)],
                        pt[:, i * NCHUNK:(i + 1) * NCHUNK],
                        start=(m == 0), stop=(m == MT - 1),
                        skip_group_check=True,
                    )

            def gt_block(q):
                # 4 G'^T m-tiles per PSUM slot (4x65 cols fit one bank)
                psg = po_pool.tile([P, NCHUNK], f32, tag="pob")
                for j in range(4):
                    m = 4 * q + j
                    nc.tensor.matmul(
                        psg[:, j * (C + 1):(j + 1) * (C + 1)],
                        xf1[:, m * P:(m + 1) * P], w2p[:],
                        start=True, stop=True,
                    )
                nc.vector.tensor_copy(
                    gt[:, 4 * q * (C + 1):4 * (q + 1) * (C + 1)],
                    psg[:, :4 * (C + 1)],
                )

            def norm_rest(t, pob):
                # pob[64] holds l (copied to lrows[t]); broadcast l to
                # pob[64:128] via col-tiled K=1 matmul, then 64-lane recip.
                nc.tensor.matmul(
                    pob[C:, :], onesb[:], lrows[t][:], start=True, stop=True,
                    tile_position=(0, C), skip_group_check=True,
                )
                lbs = out_pool.tile([C, NCHUNK], f32, tag="lbs")
                nc.vector.reciprocal(lbs[:], pob[C:, :])
                outsb = out_pool.tile([C, NCHUNK], f32)
                nc.vector.tensor_mul(outsb[:], pob[:C, :], lbs[:])
                nc.sync.dma_start(y_d[:, t * NCHUNK:(t + 1) * NCHUNK],
                                  outsb[:])

            lrows = []
            pobs = {}

            # chunk 0 with gt computation interleaved into the first groups
            pt00 = st_exp_group(0, 0)
            for q in range(4):
                gt_block(q)
            pt01 = st_exp_group(0, 1)
            for q in range(4, 8):
                gt_block(q)
            pobs[0] = po_pool.tile([P, NCHUNK], f32, tag="pob", name="pob_c0")
            pv_group(0, 0, pobs[0], pt00)
            pv_group(0, 1, pobs[0], pt01)
            for g in range(2, NG):
                pt = st_exp_group(0, g)
                pv_group(0, g, pobs[0], pt)
            lr = small_pool.tile([1, NCHUNK], bf16, tag="lrow")
            with nc.allow_low_precision(
                reason="softmax denom bf16: 0.4% scale err, gate 2e-2"
            ):
                nc.vector.tensor_copy(lr[:], pobs[0][C:C + 1, :])
            lrows.append(lr)

            for t in range(1, NT):
                pobs[t] = po_pool.tile([P, NCHUNK], f32, tag="pob", name=f"pob_c{t}")
                for g in range(NG):
                    pt = st_exp_group(t, g)
                    pv_group(t, g, pobs[t], pt)
                    if g == 2:
                        norm_rest(t - 1, pobs[t - 1])
                lr = small_pool.tile([1, NCHUNK], bf16, tag="lrow")
                with nc.allow_low_precision(
                    reason="softmax denom bf16: 0.4% scale err, gate 2e-2"
                ):
                    nc.vector.tensor_copy(lr[:], pobs[t][C:C + 1, :])
                lrows.append(lr)
            norm_rest(NT - 1, pobs[NT - 1])

    nc.compile()
    return nc


def _prep_weights(w_g, b_g, w_theta, b_theta, w_phi, b_phi, w_o, b_o):
    # wproj [65, 128]: cols 32i+c (i<3) -> theta row c; cols 96+c -> phi row c.
    # Row 64 multiplies the ones-row of xf1, i.e. carries the bias.
    wproj = np.zeros((C + 1, P), np.float32)
    for i in range(RT):
        wproj[:C, 32 * i:32 * (i + 1)] = w_theta.T
        wproj[C, 32 * i:32 * (i + 1)] = b_theta
    wproj[:C, 96:128] = w_phi.T
    wproj[C, 96:128] = b_phi

    # w2p [65, 65]: G' = [(w_o w_g) x + (w_o b_g + b_o); ones]
    w2 = w_o.astype(np.float64) @ w_g.astype(np.float64)
    bias2 = w_o.astype(np.float64) @ b_g.astype(np.float64) + b_o
    w2p = np.zeros((C + 1, C + 1), np.float32)
    w2p[:C, :C] = w2.T
    w2p[C, :C] = bias2
    w2p[C, C] = 1.0
    return wproj.astype(BF16), w2p.astype(BF16)


def kernel(x, w_g, b_g, w_theta, b_theta, w_phi, b_phi, w_o, b_o):
    import sys
    if "/opt/trn_rl_repo" not in sys.path:
        sys.path.insert(0, "/opt/trn_rl_repo")
    from concourse.bass_utils import run_bass_kernel_spmd

    if "nc" not in _cache:
        _cache["nc"] = _build_program()
    nc = _cache["nc"]

    wproj, w2p = _prep_weights(
        w_g, b_g, w_theta, b_theta, w_phi, b_phi, w_o, b_o
    )
    in_maps = []
    for b in range(B):
        xf1 = np.ones((C + 1, N), np.float32)
        xf1[:C] = np.asarray(x[b], np.float32).reshape(C, N)
        in_maps.append({
            "xf1": xf1.astype(BF16),
            "wproj": wproj,
            "w2p": w2p,
        })

    res = run_bass_kernel_spmd(nc, in_maps, core_ids=list(range(B)))
    out = np.stack([res.results[b]["y"].reshape(C, H, W) for b in range(B)])
    return np.ascontiguousarray(out.astype(np.float32))


if __name__ == "__main__":
    rng = np.random.default_rng(0)
    inputs = {
        "x": rng.normal(size=(B, C, H, W)).astype(np.float32),
        "w_g": (rng.normal(size=(CI, C)) * 0.05).astype(np.float32),
        "b_g": (rng.normal(size=(CI,)) * 0.05).astype(np.float32),
        "w_theta": (rng.normal(size=(CI, C)) * 0.05).astype(np.float32),
        "b_theta": (rng.normal(size=(CI,)) * 0.05).astype(np.float32),
        "w_phi": (rng.normal(size=(CI, C)) * 0.05).astype(np.float32),
        "b_phi": (rng.normal(size=(CI,)) * 0.05).astype(np.float32),
        "w_o": (rng.normal(size=(C, CI)) * 0.05).astype(np.float32),
        "b_o": (rng.normal(size=(C,)) * 0.05).astype(np.float32),
    }
    y = kernel(**inputs)
    print(y.shape, y.dtype)


# revision 15
# speedup vs baseline: 1.3886x; 1.0816x over previous
"""NonLocalBlock (B=8, C=64, H=W=64) on 8 Trainium2 NeuronCores.

Strategy: data-parallel over batch — core b handles batch element b fully
(no collectives). Per core, a flash-style attention over N=4096 tokens:

  xf1  = [x_b ; ones]                        [65, 4096]  (ones row folds biases)
  thphi = wproj^T @ xf1                      [128, 4096] rows 0..95 = theta
          (theta replicated 3x across partition groups for PE row tiling,
           rows 96..127 = phi)
  phi3: phi m-tiles rearranged so row-tile i holds m-tile 3g+i at
        partitions 32i..32i+32 (SBUF->SBUF DMA)
  gt   = G'^T tiles: G' = [w_o@w_g x + w_o@b_g + b_o ; ones]   [128, 65] x 32
         (ones column makes the PV matmul also produce the softmax denom)
  for each n-chunk (512 queries):
     for each m-group (3 m-tiles): S^T = phi_tile^T theta  (3x row-tiled K=32)
                                   P^T = exp(S^T)           (ScalarE, PSUM->SBUF)
                                   po += gt_m^T P^T         (K=128 accumulate)
     out = po[0:64] * (1 / po[64]) broadcast    (recip + PE bcast + DVE mult)

All matmuls bf16 (1 cyc/col on PE; fp32 would be 4x slower), accumulation and
softmax math fp32.
"""

import numpy as np
import ml_dtypes

BF16 = ml_dtypes.bfloat16

B, C, H, W = 8, 64, 64, 64
N = H * W          # 4096 tokens
CI = C // 2        # 32 intermediate channels
P = 128
NCHUNK = 512       # queries per outer chunk
NT = N // NCHUNK   # 8 chunks
MT = N // P        # 32 key tiles of 128
RT = 3             # row-tiling width (m-tiles per group)
NG = (MT + RT - 1) // RT  # 11 groups (last has 2)

_cache = {}


def _build_program():
    import concourse.mybir as mybir
    import concourse.tile as tile
    from concourse import bacc

    f32 = mybir.dt.float32
    bf16 = mybir.dt.bfloat16
    EXP = mybir.ActivationFunctionType.Exp

    nc = bacc.Bacc()
    xf1_d = nc.declare_dram_parameter("xf1", [C + 1, N], bf16, isOutput=False)
    wproj_d = nc.declare_dram_parameter("wproj", [C + 1, P], bf16, isOutput=False)
    w2p_d = nc.declare_dram_parameter("w2p", [C + 1, C + 1], bf16, isOutput=False)
    y_d = nc.declare_dram_parameter("y", [C, N], f32, isOutput=True)

    with tile.TileContext(nc) as tc:
        with (
            tc.tile_pool(name="const", bufs=1) as const_pool,
            tc.tile_pool(name="pt", bufs=3) as pt_pool,
            tc.tile_pool(name="outsb", bufs=2) as out_pool,
            tc.tile_pool(name="small", bufs=2) as small_pool,
            tc.tile_pool(name="st", bufs=2, space="PSUM") as st_pool,
            tc.tile_pool(name="po", bufs=2, space="PSUM") as po_pool,
        ):
            xf1 = const_pool.tile([C + 1, N], bf16)
            wproj = const_pool.tile([C + 1, P], bf16)
            nc.sync.dma_start(wproj[:], wproj_d[:])
            w2p = const_pool.tile([C + 1, C + 1], bf16)
            nc.sync.dma_start(w2p[:], w2p_d[:])
            onesb = const_pool.tile([1, C], bf16)
            nc.any.memset(onesb[:], 1.0)

            thphi = const_pool.tile([P, N], bf16)
            phi3 = const_pool.tile([96, NG * P], bf16)
            gt = const_pool.tile([P, MT * (C + 1)], bf16)

            def gt_block(q):
                # 4 G'^T m-tiles per PSUM slot (4x65 cols fit one bank)
                psg = po_pool.tile([P, NCHUNK], f32, tag="pob",
                                   name=f"gt_ps{q}")
                for j in range(4):
                    m = 4 * q + j
                    nc.tensor.matmul(
                        psg[:, j * (C + 1):(j + 1) * (C + 1)],
                        xf1[:, m * P:(m + 1) * P], w2p[:],
                        start=True, stop=True,
                    )
                nc.vector.tensor_copy(
                    gt[:, 4 * q * (C + 1):4 * (q + 1) * (C + 1)],
                    psg[:, :4 * (C + 1)],
                )

            def phi3_dma(i, g0, g1):
                # phi m-tiles (3g+i), g in [g0,g1) -> phi3 partitions 32i
                cnt = g1 - g0
                src = thphi[96:128].rearrange("p (m k) -> p m k", k=P)
                nc.sync.dma_start(
                    phi3[32 * i:32 * (i + 1), g0 * P:g1 * P],
                    src[:, RT * g0 + i:RT * (g1 - 1) + i + 1:RT, :],
                )

            # projections: thphi[0:96] = theta x3 replicas, [96:128] = phi.
            # xf1 DMA'd per chunk; gt blocks run as soon as their chunk lands.
            for t in range(NT):
                csl = slice(t * NCHUNK, (t + 1) * NCHUNK)
                nc.sync.dma_start(xf1[:, csl], xf1_d[:, csl])
                gt_block(t)
                ps = po_pool.tile([P, NCHUNK], f32, tag="pob")
                nc.tensor.matmul(ps[:], wproj[:], xf1[:, csl],
                                 start=True, stop=True)
                nc.vector.tensor_copy(thphi[:, csl], ps[:])
                if t == 4:
                    # m-tiles 3g+i for g<6 live in cols < 2304 (chunks 0-4)
                    for i in range(RT):
                        phi3_dma(i, 0, 6)
            for i in range(RT):
                phi3_dma(i, 6, NG if i < 2 else NG - 1)

            def st_exp_group(t, g):
                w = min(RT, MT - g * RT)
                st = st_pool.tile([P, RT * NCHUNK], f32)
                for i in range(w):
                    nc.tensor.matmul(
                        st[:, i * NCHUNK:(i + 1) * NCHUNK],
                        phi3[32 * i:32 * (i + 1), g * P:(g + 1) * P],
                        thphi[32 * i:32 * (i + 1),
                              t * NCHUNK:(t + 1) * NCHUNK],
                        start=True, stop=True,
                        tile_position=(32 * i, 0),
                    )
                pt = pt_pool.tile([P, RT * NCHUNK], bf16)
                nc.scalar.activation(pt[:, :w * NCHUNK], st[:, :w * NCHUNK],
                                     EXP)
                return pt

            def pv_group(t, g, pob, pt):
                w = min(RT, MT - g * RT)
                for i in range(w):
                    m = g * RT + i
                    nc.tensor.matmul(
                        pob[:C + 1, :],
                        gt[:, m * (C + 1):(m + 1) * (C + 1)],
                        pt[:, i * NCHUNK:(i + 1) * NCHUNK],
                        start=(m == 0), stop=(m == MT - 1),
                        skip_group_check=True,
                    )

            def norm_rest(t, pob):
                # pob[64] holds l (copied to lrows[t]); broadcast l to
                # pob[64:128] via col-tiled K=1 matmul, then 64-lane recip.
                nc.tensor.matmul(
                    pob[C:, :], onesb[:], lrows[t][:], start=True, stop=True,
                    tile_position=(0, C), skip_group_check=True,
                )
                lbs = out_pool.tile([C, NCHUNK], f32, tag="lbs")
                nc.vector.reciprocal(lbs[:], pob[C:, :])
                outsb = out_pool.tile([C, NCHUNK], f32)
                nc.vector.tensor_mul(outsb[:], pob[:C, :], lbs[:])
                nc.sync.dma_start(y_d[:, t * NCHUNK:(t + 1) * NCHUNK],
                                  outsb[:])

            lrows = []
            pobs = {}

            for t in range(NT):
                pobs[t] = po_pool.tile([P, NCHUNK], f32, tag="pob", name=f"pob_c{t}")
                for g in range(NG):
                    pt = st_exp_group(t, g)
                    pv_group(t, g, pobs[t], pt)
                    if g == 2 and t > 0:
                        norm_rest(t - 1, pobs[t - 1])
                lr = small_pool.tile([1, NCHUNK], bf16, tag="lrow")
                with nc.allow_low_precision(
                    reason="softmax denom bf16: 0.4% scale err, gate 2e-2"
                ):
                    nc.vector.tensor_copy(lr[:], pobs[t][C:C + 1, :])
                lrows.append(lr)
            norm_rest(NT - 1, pobs[NT - 1])

    nc.compile()
    return nc


def _prep_weights(w_g, b_g, w_theta, b_theta, w_phi, b_phi, w_o, b_o):
    # wproj [65, 128]: cols 32i+c (i<3) -> theta row c; cols 96+c -> phi row c.
    # Row 64 multiplies the ones-row of xf1, i.e. carries the bias.
    wproj = np.zeros((C + 1, P), np.float32)
    for i in range(RT):
        wproj[:C, 32 * i:32 * (i + 1)] = w_theta.T
        wproj[C, 32 * i:32 * (i + 1)] = b_theta
    wproj[:C, 96:128] = w_phi.T
    wproj[C, 96:128] = b_phi

    # w2p [65, 65]: G' = [(w_o w_g) x + (w_o b_g + b_o); ones]
    w2 = w_o.astype(np.float64) @ w_g.astype(np.float64)
    bias2 = w_o.astype(np.float64) @ b_g.astype(np.float64) + b_o
    w2p = np.zeros((C + 1, C + 1), np.float32)
    w2p[:C, :C] = w2.T
    w2p[C, :C] = bias2
    w2p[C, C] = 1.0
    return wproj.astype(BF16), w2p.astype(BF16)


def kernel(x, w_g, b_g, w_theta, b_theta, w_phi, b_phi, w_o, b_o):
    import sys
    if "/opt/trn_rl_repo" not in sys.path:
        sys.path.insert(0, "/opt/trn_rl_repo")
    from concourse.bass_utils import run_bass_kernel_spmd

    if "nc" not in _cache:
        _cache["nc"] = _build_program()
    nc = _cache["nc"]

    wproj, w2p = _prep_weights(
        w_g, b_g, w_theta, b_theta, w_phi, b_phi, w_o, b_o
    )
    in_maps = []
    for b in range(B):
        xf1 = np.ones((C + 1, N), np.float32)
        xf1[:C] = np.asarray(x[b], np.float32).reshape(C, N)
        in_maps.append({
            "xf1": xf1.astype(BF16),
            "wproj": wproj,
            "w2p": w2p,
        })

    res = run_bass_kernel_spmd(nc, in_maps, core_ids=list(range(B)))
    out = np.stack([res.results[b]["y"].reshape(C, H, W) for b in range(B)])
    return np.ascontiguousarray(out.astype(np.float32))


if __name__ == "__main__":
    rng = np.random.default_rng(0)
    inputs = {
        "x": rng.normal(size=(B, C, H, W)).astype(np.float32),
        "w_g": (rng.normal(size=(CI, C)) * 0.05).astype(np.float32),
        "b_g": (rng.normal(size=(CI,)) * 0.05).astype(np.float32),
        "w_theta": (rng.normal(size=(CI, C)) * 0.05).astype(np.float32),
        "b_theta": (rng.normal(size=(CI,)) * 0.05).astype(np.float32),
        "w_phi": (rng.normal(size=(CI, C)) * 0.05).astype(np.float32),
        "b_phi": (rng.normal(size=(CI,)) * 0.05).astype(np.float32),
        "w_o": (rng.normal(size=(C, CI)) * 0.05).astype(np.float32),
        "b_o": (rng.normal(size=(C,)) * 0.05).astype(np.float32),
    }
    y = kernel(**inputs)
    print(y.shape, y.dtype)


# revision 20
# speedup vs baseline: 1.4956x; 1.0770x over previous
"""NonLocalBlock (B=8, C=64, H=W=64) on 8 Trainium2 NeuronCores.

Strategy: data-parallel over batch — core b handles batch element b fully
(no collectives). Per core, a flash-style attention over N=4096 tokens:

  xf1  = [x_b ; ones]                        [65, 4096]  (ones row folds biases)
  thphi = wproj^T @ xf1                      [128, 4096] rows 0..95 = theta
          (theta replicated 3x across partition groups for PE row tiling,
           rows 96..127 = phi)
  phi3: phi m-tiles rearranged so row-tile i holds m-tile 3g+i at
        partitions 32i..32i+32 (SBUF->SBUF DMA)
  gt   = G'^T tiles: G' = [w_o@w_g x + w_o@b_g + b_o ; ones]   [128, 65] x 32
         (ones column makes the PV matmul also produce the softmax denom)
  for each n-chunk (512 queries):
     for each m-group (3 m-tiles): S^T = phi_tile^T theta  (3x row-tiled K=32)
                                   P^T = exp(S^T)           (ScalarE, PSUM->SBUF)
                                   po += gt_m^T P^T         (K=128 accumulate)
     out = po[0:64] * (1 / po[64]) broadcast    (recip + PE bcast + DVE mult)

All matmuls bf16 (1 cyc/col on PE; fp32 would be 4x slower), accumulation and
softmax math fp32.
"""

import numpy as np
import ml_dtypes

BF16 = ml_dtypes.bfloat16

B, C, H, W = 8, 64, 64, 64
N = H * W          # 4096 tokens
CI = C // 2        # 32 intermediate channels
P = 128
NCHUNK = 512       # queries per outer chunk
NT = N // NCHUNK   # 8 chunks
MT = N // P        # 32 key tiles of 128
RT = 3             # row-tiling width (m-tiles per group)
NG = (MT + RT - 1) // RT  # 11 groups (last has 2)

_cache = {}


def _build_program():
    import concourse.mybir as mybir
    import concourse.tile as tile
    from concourse import bacc

    f32 = mybir.dt.float32
    bf16 = mybir.dt.bfloat16
    EXP = mybir.ActivationFunctionType.Exp

    nc = bacc.Bacc()
    xf1_d = nc.declare_dram_parameter("xf1", [C + 1, N], bf16, isOutput=False)
    wproj_d = nc.declare_dram_parameter("wproj", [C + 1, P], bf16, isOutput=False)
    w2p_d = nc.declare_dram_parameter("w2p", [C + 1, C + 1], bf16, isOutput=False)
    y_d = nc.declare_dram_parameter("y", [C, N], f32, isOutput=True)

    with tile.TileContext(nc) as tc:
        with (
            tc.tile_pool(name="const", bufs=1) as const_pool,
            tc.tile_pool(name="pt", bufs=3) as pt_pool,
            tc.tile_pool(name="outsb", bufs=2) as out_pool,
            tc.tile_pool(name="small", bufs=2) as small_pool,
            tc.tile_pool(name="st", bufs=2, space="PSUM") as st_pool,
            tc.tile_pool(name="po", bufs=2, space="PSUM") as po_pool,
        ):
            xf1 = const_pool.tile([C + 1, N], bf16)
            wproj = const_pool.tile([C + 1, P], bf16)
            nc.sync.dma_start(wproj[:], wproj_d[:])
            w2p = const_pool.tile([C + 1, C + 1], bf16)
            nc.sync.dma_start(w2p[:], w2p_d[:])
            onesb = const_pool.tile([1, C], bf16)
            nc.any.memset(onesb[:], 1.0)
            warm = const_pool.tile([1, NCHUNK], bf16)
            nc.any.memset(warm[:], 1.0)

            thphi = const_pool.tile([P, N], bf16)
            phi3 = const_pool.tile([96, NG * P], bf16)
            gt = const_pool.tile([P, MT * (C + 1)], bf16)

            def gt_block(q):
                # 4 G'^T m-tiles per PSUM slot (4x65 cols fit one bank)
                psg = po_pool.tile([P, NCHUNK], f32, tag="pob",
                                   name=f"gt_ps{q}")
                for j in range(4):
                    m = 4 * q + j
                    nc.tensor.matmul(
                        psg[:, j * (C + 1):(j + 1) * (C + 1)],
                        xf1[:, m * P:(m + 1) * P], w2p[:],
                        start=True, stop=True,
                    )
                nc.vector.tensor_copy(
                    gt[:, 4 * q * (C + 1):4 * (q + 1) * (C + 1)],
                    psg[:, :4 * (C + 1)],
                )

            def phi3_dma(i, g0, g1):
                # phi m-tiles (3g+i), g in [g0,g1) -> phi3 partitions 32i
                src = thphi[96:128].rearrange("p (m k) -> p m k", k=P)
                nc.gpsimd.dma_start(
                    phi3[32 * i:32 * (i + 1), g0 * P:g1 * P],
                    src[:, RT * g0 + i:RT * (g1 - 1) + i + 1:RT, :],
                )

            # PE warmup: ~4us of dummy matmuls so the HAM clock gate opens
            # (K=8/8) before the real pipeline starts.
            wpsum = po_pool.tile([P, NCHUNK], f32, tag="pob", name="warmps")
            for _ in range(6):
                nc.tensor.matmul(wpsum[:C, :], onesb[:], warm[:],
                                 start=True, stop=True)

            # all xf1 chunk DMAs issued upfront on both queue engines
            for t in range(NT):
                csl = slice(t * NCHUNK, (t + 1) * NCHUNK)
                eng = nc.sync if t % 2 == 0 else nc.gpsimd
                eng.dma_start(xf1[:, csl], xf1_d[:, csl])

            # projections: thphi[0:96] = theta x3 replicas, [96:128] = phi;
            # gt blocks run as each chunk lands.
            for t in range(NT):
                csl = slice(t * NCHUNK, (t + 1) * NCHUNK)
                gt_block(t)
                ps = po_pool.tile([P, NCHUNK], f32, tag="pob")
                nc.tensor.matmul(ps[:], wproj[:], xf1[:, csl],
                                 start=True, stop=True)
                nc.vector.tensor_copy(thphi[:, csl], ps[:])
                if t == 1:
                    for i in range(RT):
                        phi3_dma(i, 0, 2)   # m-tiles <8: chunks 0-1
                if t == 4:
                    for i in range(RT):
                        phi3_dma(i, 2, 6)   # m-tiles <18: chunks 0-4
            for i in range(RT):
                phi3_dma(i, 6, NG if i < 2 else NG - 1)

            def st_exp_group(t, g):
                w = min(RT, MT - g * RT)
                st = st_pool.tile([P, RT * NCHUNK], f32)
                for i in range(w):
                    nc.tensor.matmul(
                        st[:, i * NCHUNK:(i + 1) * NCHUNK],
                        phi3[32 * i:32 * (i + 1), g * P:(g + 1) * P],
                        thphi[32 * i:32 * (i + 1),
                              t * NCHUNK:(t + 1) * NCHUNK],
                        start=True, stop=True,
                        tile_position=(32 * i, 0),
                    )
                pt = pt_pool.tile([P, RT * NCHUNK], bf16)
                nc.scalar.activation(pt[:, :w * NCHUNK], st[:, :w * NCHUNK],
                                     EXP)
                return pt

            def pv_group(t, g, pob, pt):
                w = min(RT, MT - g * RT)
                for i in range(w):
                    m = g * RT + i
                    nc.tensor.matmul(
                        pob[:C + 1, :],
                        gt[:, m * (C + 1):(m + 1) * (C + 1)],
                        pt[:, i * NCHUNK:(i + 1) * NCHUNK],
                        start=(m == 0), stop=(m == MT - 1),
                        skip_group_check=True,
                    )

            def norm_rest(t, pob):
                # pob[64] holds l (copied to lrows[t]); broadcast l to
                # pob[64:128] via col-tiled K=1 matmul, then divide.
                nc.tensor.matmul(
                    pob[C:, :], onesb[:], lrows[t][:], start=True, stop=True,
                    tile_position=(0, C), skip_group_check=True,
                )
                lbs = out_pool.tile([C, NCHUNK], f32, tag="lbs")
                nc.vector.reciprocal(lbs[:], pob[C:, :])
                outsb = out_pool.tile([C, NCHUNK], f32)
                nc.vector.tensor_mul(outsb[:], pob[:C, :], lbs[:])
                nc.sync.dma_start(y_d[:, t * NCHUNK:(t + 1) * NCHUNK],
                                  outsb[:])

            def lrow_copy(t):
                lr = small_pool.tile([1, NCHUNK], bf16, tag="lrow",
                                     name=f"lrow{t}")
                with nc.allow_low_precision(
                    reason="softmax denom bf16: 0.4% scale err, gate 2e-2"
                ):
                    nc.vector.tensor_copy(lr[:], pobs[t][C:C + 1, :])
                lrows.append(lr)

            lrows = []
            pobs = {}
            # PV runs one group behind S^T/exp so a PV group never sits
            # between the row-tiled S^T matmuls feeding the next exp.
            pend = None
            for t in range(NT):
                pobs[t] = po_pool.tile([P, NCHUNK], f32, tag="pob",
                                       name=f"pob_c{t}")
                for g in range(NG):
                    pt = st_exp_group(t, g)
                    if pend is not None:
                        pv_group(pend[0], pend[1], pobs[pend[0]], pend[2])
                        if pend[1] == NG - 1:
                            lrow_copy(pend[0])
                    pend = (t, g, pt)
                    if g == 2 and t > 0:
                        norm_rest(t - 1, pobs[t - 1])
            pv_group(pend[0], pend[1], pobs[pend[0]], pend[2])
            lrow_copy(pend[0])
            norm_rest(NT - 1, pobs[NT - 1])

    nc.compile()
    return nc


def _prep_weights(w_g, b_g, w_theta, b_theta, w_phi, b_phi, w_o, b_o):
    # wproj [65, 128]: cols 32i+c (i<3) -> theta row c; cols 96+c -> phi row c.
    # Row 64 multiplies the ones-row of xf1, i.e. carries the bias.
    wproj = np.zeros((C + 1, P), np.float32)
    for i in range(RT):
        wproj[:C, 32 * i:32 * (i + 1)] = w_theta.T
        wproj[C, 32 * i:32 * (i + 1)] = b_theta
    wproj[:C, 96:128] = w_phi.T
    wproj[C, 96:128] = b_phi

    # w2p [65, 65]: G' = [(w_o w_g) x + (w_o b_g + b_o); ones]
    w2 = w_o.astype(np.float64) @ w_g.astype(np.float64)
    bias2 = w_o.astype(np.float64) @ b_g.astype(np.float64) + b_o
    w2p = np.zeros((C + 1, C + 1), np.float32)
    w2p[:C, :C] = w2.T
    w2p[C, :C] = bias2
    w2p[C, C] = 1.0
    return wproj.astype(BF16), w2p.astype(BF16)


def kernel(x, w_g, b_g, w_theta, b_theta, w_phi, b_phi, w_o, b_o):
    import sys
    if "/opt/trn_rl_repo" not in sys.path:
        sys.path.insert(0, "/opt/trn_rl_repo")
    from concourse.bass_utils import run_bass_kernel_spmd

    if "nc" not in _cache:
        _cache["nc"] = _build_program()
    nc = _cache["nc"]

    wproj, w2p = _prep_weights(
        w_g, b_g, w_theta, b_theta, w_phi, b_phi, w_o, b_o
    )
    in_maps = []
    for b in range(B):
        xf1 = np.ones((C + 1, N), np.float32)
        xf1[:C] = np.asarray(x[b], np.float32).reshape(C, N)
        in_maps.append({
            "xf1": xf1.astype(BF16),
            "wproj": wproj,
            "w2p": w2p,
        })

    res = run_bass_kernel_spmd(nc, in_maps, core_ids=list(range(B)))
    out = np.stack([res.results[b]["y"].reshape(C, H, W) for b in range(B)])
    return np.ascontiguousarray(out.astype(np.float32))


if __name__ == "__main__":
    rng = np.random.default_rng(0)
    inputs = {
        "x": rng.normal(size=(B, C, H, W)).astype(np.float32),
        "w_g": (rng.normal(size=(CI, C)) * 0.05).astype(np.float32),
        "b_g": (rng.normal(size=(CI,)) * 0.05).astype(np.float32),
        "w_theta": (rng.normal(size=(CI, C)) * 0.05).astype(np.float32),
        "b_theta": (rng.normal(size=(CI,)) * 0.05).astype(np.float32),
        "w_phi": (rng.normal(size=(CI, C)) * 0.05).astype(np.float32),
        "b_phi": (rng.normal(size=(CI,)) * 0.05).astype(np.float32),
        "w_o": (rng.normal(size=(C, CI)) * 0.05).astype(np.float32),
        "b_o": (rng.normal(size=(C,)) * 0.05).astype(np.float32),
    }
    y = kernel(**inputs)
    print(y.shape, y.dtype)
